# revision 1
# baseline (speedup 1.0000x reference)
"""Trainium2 Bass kernel for nn_Block_local (local windowed attention block).

Per-batch computation (reference semantics):
    q = LN(query + query_embed) -> 1x1 conv wq     (LN over channels, shared g/b)
    k = LN(key + key_embed)     -> 1x1 conv wk
    v = wv @ key + bv                               (conv on the RAW key)
    w[n, j] = sum_c q[c,n] * k_pad[c, n+j-pad]      j in [0, kH)
    w = softmax_j(w) * C**-0.5
    attn[c,n] = sum_j w[n,j] * v_pad[c, n+j-pad]
    x = query + attn
    x = x + MLP(LN2(x))                             (MLP: gelu(x@w1+b1)@w2+b2)

Sharding: data-parallel over batch B=8 across the 8 NeuronCores (one batch
per core); every core runs an identical program on its own batch slice.

Device-side algebra (host pre-folds all affine pieces):
  - LN gain/bias + q/k conv weights/biases fold into one similarity matrix
    in augmented space z = [xhat; 1]:  G = z_q^T (Aq^T Ak) z_k with
    Aq = [wq*g | wq@b_norm + bq].  Device computes kz = Mz @ z_k (lhsT=MzT),
    then banded G blocks via z_q^T @ kz windows.
  - Window = banded gram per 128-row block: affine_select band mask, band
    softmax (exp w/ accum_out), PE transpose of the weights, banded weighting
    matmul vs vT with an on-the-fly DMA halo for the upper band piece.
  - v bias bv rides the residual: softmax rows sum to `scale`, so the host
    adds scale*bv into query^T (the qTb input).
  - MLP phase re-loads the residual from a DRAM bounce (x2d), does LN2,
    PE-transposes to C layout, mm1 + Gelu(bias=c1), and mm2 emitting
    T-layout output directly with b2 via an augmented ones-row matmul and
    the final residual add fused behind it.

All big matmuls use float32r (PE fast path: 1 cycle/row vs 4 for fp32;
~1.4e-4 relative error, verified on HW). HW Gelu == erf gelu (verified).
"""

import os
from contextlib import ExitStack

import numpy as np

import concourse.bass as bass
import concourse.tile as tile
from concourse import bacc, mybir
from concourse.bass_utils import run_bass_kernel_spmd
from concourse.masks import make_identity
import ml_dtypes

f32 = mybir.dt.float32
f32r = mybir.dt.float32r
bf16 = mybir.dt.bfloat16
AF = mybir.ActivationFunctionType
ALU = mybir.AluOpType
AX = mybir.AxisListType

P = 128          # partitions
C = 512          # channels
H = 4 * C        # mlp hidden
EPS = 1e-5
NEG = -1e30

CT = C // P      # channel tiles (4)
HT = H // P      # mlp hidden tiles (16)

ts = bass.ts


def build_block_kernel(nc, N, KH, gelu_func=AF.Gelu, reps=1):
    """Emit the tile program. N = sequence length, KH = window size (odd)."""
    NT = N // P               # n tiles
    PADW = KH // 2            # 4
    W = P + KH - 1            # band tile width (136)
    NCH = N // 512            # 512-wide column chunks
    scale = C ** -0.5

    # ---- DRAM I/O ----
    dI = {}
    for nm, shp, dt in [
        ("qT", [N, C], f32), ("qeT", [N, C], f32), ("kT", [N, C], f32),
        ("keT", [N, C], f32), ("qTb", [N, C], f32), ("keyC", [C, N], f32r),
        ("MzT", [C + 1, C + 1], bf16), ("wvT", [C, C], f32r),
        ("W1p", [C, H], f32r), ("w2", [H, C], bf16),
        ("c1t", [P, HT], f32), ("b2r", [1, C], f32r), ("onesr", [1, 512], f32r),
    ]:
        dI[nm] = nc.dram_tensor(nm, shp, dt, kind="ExternalInput").ap()
    outT = nc.dram_tensor("outT", [N, C], f32, kind="ExternalOutput").ap()
    x2d = nc.dram_tensor("x2d", [N, C], f32).ap()  # internal residual bounce

    with tile.TileContext(nc, pool_alloc_mode="queue") as tc, ExitStack() as ctx:
        # ---------- long-lived pools ----------
        psum = ctx.enter_context(tc.tile_pool(name="psum", bufs=6, space="PSUM"))
        _ctr = [0]

        def pt(shape, tag="ps", bufs=None, dt=f32):
            _ctr[0] += 1
            return psum.tile(shape, dt, tag=tag, name=f"pst{_ctr[0]}", bufs=bufs)

        const = ctx.enter_context(tc.tile_pool(name="const", bufs=1))
        stat_p = ctx.enter_context(tc.tile_pool(name="stat", bufs=8))
        work_p = ctx.enter_context(tc.tile_pool(name="work", bufs=6))

        ident = const.tile([P, P], f32)
        make_identity(nc, ident)
        ident_bf = const.tile([P, P], bf16)
        make_identity(nc, ident_bf)
        ones_row = const.tile([1, 512], f32r)
        nc.sync.dma_start(ones_row, dI["onesr"])
        ones_bf = const.tile([1, 512], bf16)
        nc.vector.memset(ones_bf, 1.0)
        eps_col = const.tile([P, 1], f32)
        nc.vector.memset(eps_col, EPS)
        c1_sb = const.tile([P, HT], f32)
        nc.sync.dma_start(c1_sb, dI["c1t"])
        b2_sb = const.tile([1, C], f32r)
        nc.sync.dma_start(b2_sb, dI["b2r"])

        # ---------- helpers ----------
        def emit_once():
            def load_quad(src, q4, tag):
                """One DMA pulling 4 n-tiles of a T-layout DRAM tensor."""
                t = ldq_pool.tile([P, 4, C], f32, tag=tag, bufs=2)
                nc.sync.dma_start(
                    t, src[ts(q4, 4 * P), :].rearrange("(d p) c -> p d c", p=P))
                return t

            def ln_of(s, odt=f32):
                st6 = stat_p.tile([P, 6], f32, tag="st6")
                nc.vector.bn_stats(st6, s)
                mv = stat_p.tile([P, 2], f32, tag="mv")
                nc.vector.bn_aggr(mv, st6)
                sd = stat_p.tile([P, 1], f32, tag="sd")
                nc.scalar.activation(sd, mv[:, 1:2], AF.Sqrt, bias=eps_col, scale=1.0)
                r = stat_p.tile([P, 1], f32, tag="r")
                nc.vector.reciprocal(r, sd)
                nmr = stat_p.tile([P, 1], f32, tag="nmr")
                nc.vector.tensor_scalar(out=nmr, in0=mv[:, 0:1], scalar1=r, scalar2=-1.0,
                                        op0=ALU.mult, op1=ALU.mult)
                xh = work_p.tile([P, C], odt, tag="xh")
                nc.scalar.activation(xh, s, AF.Identity, bias=nmr, scale=r)
                return xh

            def ln_xhat_quad(src_a, src_b, q4):
                a = load_quad(src_a, q4, "ld_a")
                b = load_quad(src_b, q4, "ld_b")
                outs = []
                for d in range(4):
                    s = work_p.tile([P, C], f32, tag="sum", bufs=4)
                    nc.gpsimd.tensor_add(s, a[:, d, :], b[:, d, :])
                    outs.append(ln_of(s, odt=bf16))
                return outs

            def transpose_quad(xh4, ct, dst, dst_cols, evict="vector"):
                """Transpose the ct-th c-block of 4 T-tiles into dst[:, dst_cols]."""
                dt_ = xh4[0].dtype
                idn = ident_bf if dt_ == bf16 else ident
                ps_t = pt([P, 512], dt=dt_)
                for d, xh in enumerate(xh4):
                    nc.tensor.transpose(ps_t[:, ts(d, P)], xh[:, ts(ct, P)], idn)
                if evict == "vector":
                    nc.vector.tensor_copy(dst[:, dst_cols], ps_t)
                elif evict == "scalar":
                    nc.scalar.copy(dst[:, dst_cols], ps_t)
                else:  # alternate
                    if ct % 2 == 0:
                        nc.vector.tensor_copy(dst[:, dst_cols], ps_t)
                    else:
                        nc.scalar.copy(dst[:, dst_cols], ps_t)

            # LIFO pool discipline: longer-lived pools allocated first.
            kz_pool = tc.alloc_tile_pool(name="kzp", bufs=1)
            aqr_pool = tc.alloc_tile_pool(name="aqrp", bufs=1)
            ldq_pool = tc.alloc_tile_pool(name="ldqp", bufs=2)

            skip_kq = bool(os.environ.get("KSKIP_KQ"))
            # ================= k side: xhat_k -> akrC -> kz =================
            mz_pool = tc.alloc_tile_pool(name="mzp", bufs=1)
            MzT_sb = []
            for kb in range(CT):
                t = mz_pool.tile([P, C + 1], bf16, name=f"MzT{kb}")
                nc.sync.dma_start(t, dI["MzT"][ts(kb, P), :])
                MzT_sb.append(t)
            MzT_last = mz_pool.tile([1, C + 1], bf16)
            nc.sync.dma_start(MzT_last, dI["MzT"][C:C + 1, :])

            akr_pool = tc.alloc_tile_pool(name="akrp", bufs=1)
            akrC = [akr_pool.tile([P, N], bf16, name=f"akr{ct}") for ct in range(CT)]
            for q4 in range(NT // 4):
                if skip_kq:
                    break
                quad = ln_xhat_quad(dI["kT"], dI["keT"], q4)
                for ct in range(CT):
                    transpose_quad(quad, ct, akrC[ct], ts(q4, 512), evict="alt")

            kz_sb = [kz_pool.tile([P, N], bf16, name=f"kz{m}") for m in range(CT)]
            kz_last = kz_pool.tile([1, N], bf16)
            for m in range(CT + 1):
                if os.environ.get("KSKIP_KZ"):
                    break
                small = m == CT
                m_sl = slice(C, C + 1) if small else ts(m, P)
                for ch in range(NCH):
                    ps_k = pt([1 if small else P, 512],
                              tag="ps_s" if small else "ps", bufs=2 if small else None)
                    for kb in range(CT):
                        nc.tensor.matmul(ps_k, MzT_sb[kb][:, m_sl],
                                         akrC[kb][:, ts(ch, 512)],
                                         start=(kb == 0), stop=False)
                    nc.tensor.matmul(ps_k, MzT_last[:, m_sl], ones_bf,
                                     start=False, stop=True)
                    dst = kz_last if small else kz_sb[m]
                    nc.scalar.copy(dst[:, ts(ch, 512)], ps_k)
            akr_pool.release()
            mz_pool.release()

            # ================= q side: xhat_q -> aqrC =================
            aqrC = [aqr_pool.tile([P, N], bf16, name=f"aqr{ct}") for ct in range(CT)]
            for q4 in range(NT // 4):
                if skip_kq:
                    break
                quad = ln_xhat_quad(dI["qT"], dI["qeT"], q4)
                for ct in range(CT):
                    transpose_quad(quad, ct, aqrC[ct], ts(q4, 512), evict="alt")

            ldq_pool.release()

            # --- MLP weight prefetch: pools allocated before the attention
            # pools; DMAs issued on the gpsimd (SWDGE) queue so they do not
            # contend with attention-critical HWDGE loads.
            x2r_pool = tc.alloc_tile_pool(name="x2rp", bufs=8)
            mlpw = tc.alloc_tile_pool(name="mlpw", bufs=1)
            W1p_sb = [mlpw.tile([P, H], f32r, name=f"W1p{kb}")
                      for kb in range(CT)]
            w2_sb = []
            w2_tiles = [mlpw.tile([P, 4, C], bf16, name=f"w2_{g4}")
                        for g4 in range(HT // 4)]
            weight_dmas = []
            for kb in range(CT):
                for col in range(4):
                    weight_dmas.append((W1p_sb[kb][:, ts(col, 512)],
                                        dI["W1p"][ts(kb, P), ts(col, 512)]))
            for g4 in range(HT // 4):
                weight_dmas.append((
                    w2_tiles[g4],
                    dI["w2"][ts(g4, 4 * P), :].rearrange("(d p) c -> p d c", p=P)))
                for d in range(4):
                    w2_sb.append(w2_tiles[g4][:, d, :])

            # ================= vT conv (streamed keyC, chunk-interleaved) ========
            wv_pool = tc.alloc_tile_pool(name="wvp", bufs=1)
            key_pool = tc.alloc_tile_pool(name="keyp", bufs=6)
            vt_pool = tc.alloc_tile_pool(name="vtp", bufs=9)
            x2_pool = tc.alloc_tile_pool(name="x2p", bufs=4)
            attn_p = tc.alloc_tile_pool(name="attnp", bufs=3)
            halo_p = tc.alloc_tile_pool(name="halop", bufs=3)
            wvT_sb = []
            for kb in range(CT):
                t = wv_pool.tile([P, C], f32r, name=f"wvT{kb}")
                nc.sync.dma_start(t, dI["wvT"][ts(kb, P), :])
                wvT_sb.append(t)
            vT_tiles = []

            def emit_v_chunk(ch):
                kc = []
                for kb in range(CT):
                    t = key_pool.tile([P, 512], f32r, tag="keyc")
                    nc.sync.dma_start(t, dI["keyC"][ts(kb, P), ts(ch, 512)])
                    kc.append(t)
                for sub in range(4):
                    ps_v = pt([P, C])
                    for kb in range(CT):
                        nc.tensor.matmul(ps_v, kc[kb][:, ts(sub, P)], wvT_sb[kb],
                                         start=(kb == 0), stop=(kb == CT - 1))
                    vt = vt_pool.tile([P, C], f32r, tag="vt")
                    nc.scalar.copy(vt, ps_v)
                    vT_tiles.append(vt)

            emit_v_chunk(0)

            # ================= attention =================

            for g in range(NT // 4):
                if g + 1 < NCH:
                    emit_v_chunk(g + 1)
                qtbq = [None, None]
                for h in range(2):
                    qtbq[h] = x2_pool.tile([P, 2, C], f32, tag="qtb",
                                           name=f"qtbq{g}_{h}", bufs=2)
                    nc.sync.dma_start(
                        qtbq[h], dI["qTb"][ts(g * 2 + h, 2 * P), :].rearrange(
                            "(d p) c -> p d c", p=P))
                # ---- pass 1: banded gram + band softmax (PE: grams only)
                wns = []
                for b in range(4):
                    nb = g * 4 + b
                    fl = PADW if nb == 0 else 0
                    fh = W - PADW if nb == NT - 1 else W
                    wvd = fh - fl
                    plo = nb * P - PADW + fl
                    ps_g = pt([P, W])
                    for kb in range(CT):
                        nc.tensor.matmul(ps_g[:, fl:fh], aqrC[kb][:, ts(nb, P)],
                                         kz_sb[kb][:, plo:plo + wvd],
                                         start=(kb == 0), stop=False)
                    nc.tensor.matmul(ps_g[:, fl:fh], ones_bf[:, 0:P],
                                     kz_last[:, plo:plo + wvd],
                                     start=False, stop=True)
                    gs = attn_p.tile([P, W], f32, tag="gs", bufs=5)
                    if fl > 0:
                        nc.vector.memset(gs[:, 0:fl], 0.0)
                    if fh < W:
                        nc.vector.memset(gs[:, fh:W], 0.0)
                    nc.scalar.copy(gs[:, fl:fh], ps_g[:, fl:fh])
                    # band: keep where 0 <= f - p <= KH-1
                    nc.gpsimd.affine_select(out=gs, in_=gs, pattern=[[1, W]],
                                            base=0, channel_multiplier=-1,
                                            compare_op=ALU.is_ge, fill=NEG)
                    nc.gpsimd.affine_select(out=gs, in_=gs, pattern=[[-1, W]],
                                            base=KH - 1, channel_multiplier=1,
                                            compare_op=ALU.is_ge, fill=NEG)
                    nmx = stat_p.tile([P, 1], f32, tag="nmx")
                    nc.vector.reduce_max(out=nmx, in_=gs, axis=AX.X, negate=True)
                    ge = attn_p.tile([P, W], f32, tag="ge", bufs=5)
                    esum = stat_p.tile([P, 1], f32, tag="esum")
                    nc.scalar.activation(ge, gs, AF.Exp, bias=nmx, scale=1.0,
                                         accum_out=esum)
                    rsc = stat_p.tile([P, 1], f32, tag="rsc")
                    nc.vector.reciprocal(rsc, esum)
                    wn = attn_p.tile([P, W], f32, tag="wn", bufs=5)
                    nc.vector.tensor_scalar(out=wn, in0=ge, scalar1=rsc,
                                            scalar2=scale, op0=ALU.mult,
                                            op1=ALU.mult)
                    wns.append(wn)
                # ---- pass 2: PE transposes of the band pieces
                wbs = []
                for b in range(4):
                    nb = g * 4 + b
                    wn = wns[b]
                    ps_w = pt([P, 3 * P])
                    if nb > 0:
                        nc.tensor.transpose(ps_w[0:PADW, 0:P], wn[:, 0:PADW],
                                            ident)
                    nc.tensor.transpose(ps_w[:, P:2 * P], wn[:, PADW:PADW + P],
                                        ident)
                    if nb < NT - 1:
                        nc.tensor.transpose(ps_w[0:PADW, 2 * P:3 * P],
                                            wn[:, PADW + P:W], ident)
                    wbB = attn_p.tile([P, P], f32r, tag="wbB", bufs=5)
                    nc.vector.tensor_copy(wbB, ps_w[:, P:2 * P])
                    wbA = wbC = None
                    if nb > 0:
                        wbA = attn_p.tile([PADW, P], f32r, tag="wbA", bufs=5)
                        nc.scalar.copy(wbA, ps_w[0:PADW, 0:P])
                    if nb < NT - 1:
                        wbC = attn_p.tile([PADW, P], f32r, tag="wbC", bufs=5)
                        nc.scalar.copy(wbC, ps_w[0:PADW, 2 * P:3 * P])
                    wbs.append((wbA, wbB, wbC))
                # ---- pass 3: banded weighting + residual
                for b in range(4):
                    nb = g * 4 + b
                    wbA, wbB, wbC = wbs[b]
                    ps_a = pt([P, C])
                    first = True
                    if nb > 0:
                        halo = halo_p.tile([PADW, C], f32r, tag="halo")
                        nc.sync.dma_start(halo, vT_tiles[nb - 1][P - PADW:P, :])
                        nc.tensor.matmul(ps_a, wbA, halo, start=True, stop=False)
                        first = False
                    nc.tensor.matmul(ps_a, wbB, vT_tiles[nb],
                                     start=first, stop=(nb == NT - 1))
                    if nb < NT - 1:
                        nc.tensor.matmul(ps_a, wbC, vT_tiles[nb + 1][0:PADW, :],
                                         start=False, stop=True)
                    x2 = x2_pool.tile([P, C], f32, tag="x2")
                    nc.vector.tensor_add(x2, ps_a, qtbq[b // 2][:, b % 2, :])
                    nc.sync.dma_start(x2d[ts(nb, P), :], x2)
                # interleave a slice of the MLP weight prefetch
                nper = (len(weight_dmas) + NT // 4 - 1) // (NT // 4)
                for dst, src in weight_dmas[g * nper:(g + 1) * nper]:
                    nc.sync.dma_start(dst, src)

            halo_p.release()
            attn_p.release()
            x2_pool.release()
            vt_pool.release()
            key_pool.release()
            wv_pool.release()

            if os.environ.get("KSKIP_MLP"):
                mlpw.release()
                x2r_pool.release()
                aqr_pool.release()
                kz_pool.release()
                return
            # ================= MLP phase =================
            x2q_pre = []
            for ch in range(min(2, NCH)):
                x2q = x2r_pool.tile([P, 4, C], f32, tag="x2r",
                                    name=f"x2q{ch}", bufs=2)
                nc.sync.dma_start(
                    x2q, x2d[ts(ch, 4 * P), :].rearrange("(d p) c -> p d c", p=P))
                x2q_pre.append(x2q)

            xh2c_pool = tc.alloc_tile_pool(name="xh2cp", bufs=8)
            hg_pool = tc.alloc_tile_pool(name="hgp", bufs=18)
            fin_pool = tc.alloc_tile_pool(name="finp", bufs=4)

            for ch in range(NCH):
                if ch < len(x2q_pre):
                    x2q = x2q_pre[ch]
                else:
                    x2q = x2r_pool.tile([P, 4, C], f32, tag="x2r",
                                        name=f"x2ql{ch}", bufs=2)
                    nc.sync.dma_start(
                        x2q, x2d[ts(ch, 4 * P), :].rearrange(
                            "(d p) c -> p d c", p=P))
                x2c = [x2q[:, d, :] for d in range(4)]
                xh2 = [ln_of(t) for t in x2c]
                xh2c = []
                for ct in range(CT):
                    dst = xh2c_pool.tile([P, 512], f32r, tag="xh2c")
                    transpose_quad(xh2, ct, dst, slice(0, 512))
                    xh2c.append(dst)
                # mm1 + gelu
                hg = []
                for m in range(HT):
                    ps_h = pt([P, 512])
                    for kb in range(CT):
                        nc.tensor.matmul(ps_h, W1p_sb[kb][:, ts(m, P)], xh2c[kb],
                                         start=(kb == 0), stop=(kb == CT - 1))
                    hgt = hg_pool.tile([P, 512], bf16, tag="hg")
                    nc.scalar.activation(hgt, ps_h, gelu_func,
                                         bias=c1_sb[:, m:m + 1], scale=1.0)
                    hg.append(hgt)
                # mm2 (T-layout out) + b2 aug + residual
                for sub in range(4):
                    nb = ch * 4 + sub
                    ps_o = pt([P, C])
                    for kb in range(HT):
                        nc.tensor.matmul(ps_o, hg[kb][:, ts(sub, P)], w2_sb[kb],
                                         start=(kb == 0), stop=False)
                    nc.tensor.matmul(ps_o, ones_row[:, 0:P], b2_sb,
                                     start=False, stop=True)
                    fin = fin_pool.tile([P, C], f32, tag="fin")
                    nc.vector.tensor_add(fin, ps_o, x2c[sub])
                    nc.sync.dma_start(outT[ts(nb, P), :], fin)

            fin_pool.release()
            hg_pool.release()
            xh2c_pool.release()
            mlpw.release()
            x2r_pool.release()
            aqr_pool.release()
            kz_pool.release()

        for _rep in range(reps):
            emit_once()

    return dI, outT


_CACHE = {}


def _get_compiled(N, KH, gelu_func=AF.Gelu, reps=1):
    key = (N, KH, str(gelu_func), reps)
    if key not in _CACHE:
        nc = bacc.Bacc("TRN2", target_bir_lowering=False, debug=False,
                       enable_asserts=False)
        build_block_kernel(nc, N, KH, gelu_func, reps=reps)
        nc.compile()
        _CACHE[key] = nc
    return _CACHE[key]


def host_prep(inputs, N, KH):
    """Fold weights and build the per-core input maps."""
    q = np.asarray(inputs["query"], np.float32)
    k = np.asarray(inputs["key"], np.float32)
    qe = np.asarray(inputs["query_embed"], np.float32)
    ke = np.asarray(inputs["key_embed"], np.float32)
    wq = np.asarray(inputs["wq"], np.float32)
    bq = np.asarray(inputs["bq"], np.float32)
    wk = np.asarray(inputs["wk"], np.float32)
    bk = np.asarray(inputs["bk"], np.float32)
    wv = np.asarray(inputs["wv"], np.float32)
    bv = np.asarray(inputs["bv"], np.float32)
    g = np.asarray(inputs["g_norm"], np.float32)
    b = np.asarray(inputs["b_norm"], np.float32)
    g2 = np.asarray(inputs["g_norm2"], np.float32)
    b2n = np.asarray(inputs["b_norm2"], np.float32)
    w1 = np.asarray(inputs["w1"], np.float32)
    b1 = np.asarray(inputs["b1"], np.float32)
    w2 = np.asarray(inputs["w2"], np.float32)
    b2 = np.asarray(inputs["b2"], np.float32)

    Bsz = q.shape[0]
    scale = C ** -0.5

    Aq = np.concatenate([wq * g[None, :], (wq @ b + bq)[:, None]], axis=1)
    Ak = np.concatenate([wk * g[None, :], (wk @ b + bk)[:, None]], axis=1)
    MzT = np.ascontiguousarray(Ak.T @ Aq).astype(ml_dtypes.bfloat16)

    W1p = np.ascontiguousarray(w1 * g2[:, None])
    c1 = b2n @ w1 + b1
    c1t = np.ascontiguousarray(c1.reshape(HT, P).T)
    shared = {
        "MzT": MzT,
        "wvT": np.ascontiguousarray(wv.T),
        "W1p": W1p,
        "w2": np.ascontiguousarray(w2).astype(ml_dtypes.bfloat16),
        "c1t": c1t,
        "b2r": np.ascontiguousarray(b2[None, :]),
        "onesr": np.ones((1, 512), np.float32),
    }
    in_maps = []
    for i in range(Bsz):
        m = dict(shared)
        m["qT"] = np.ascontiguousarray(q[i].T)
        m["qeT"] = np.ascontiguousarray(qe[i].T)
        m["kT"] = np.ascontiguousarray(k[i].T)
        m["keT"] = np.ascontiguousarray(ke[i].T)
        m["qTb"] = np.ascontiguousarray(q[i].T + scale * bv[None, :])
        m["keyC"] = np.ascontiguousarray(k[i])
        in_maps.append(m)
    return in_maps


def kernel(**inputs):
    q = np.asarray(inputs["query"])
    Bsz, Cin, N = q.shape
    assert Cin == C, f"built for C={C}"
    KH = int(inputs["kH"])
    nc = _get_compiled(N, KH)
    in_maps = host_prep(inputs, N, KH)
    core_ids = list(range(len(in_maps)))
    res = run_bass_kernel_spmd(nc, in_maps, core_ids)
    out = np.stack([np.ascontiguousarray(r["outT"].T) for r in res.results], axis=0)
    return out.astype(np.float32)


if __name__ == "__main__":
    _get_compiled(2048, 9)
    print("built + compiled OK")



# revision 82
# speedup vs baseline: 2.4295x; 2.4295x over previous
"""Trainium2 Bass kernel for nn_Block_local (local windowed attention block).

Per-batch computation (reference semantics):
    q = LN(query + query_embed) -> 1x1 conv wq     (LN over channels, shared g/b)
    k = LN(key + key_embed)     -> 1x1 conv wk
    v = wv @ key + bv                               (conv on the RAW key)
    w[n, j] = sum_c q[c,n] * k_pad[c, n+j-pad]      j in [0, kH)
    w = softmax_j(w) * C**-0.5
    attn[c,n] = sum_j w[n,j] * v_pad[c, n+j-pad]
    x = query + attn
    x = x + MLP(LN2(x))                             (MLP: gelu(x@w1+b1)@w2+b2)

Sharding: data-parallel over batch B=8 across the 8 NeuronCores (one batch
per core); every core runs an identical program on its own batch slice.

Device-side algebra (host pre-folds all affine pieces):
  - LN gain/bias + q/k conv weights/biases fold into one similarity matrix
    in augmented space z = [xhat; 1]:  G = z_q^T (Aq^T Ak) z_k.  Mz is
    host-scaled by MZS to dodge fp8 subnormals; 1/MZS rides the exp scale.
    kz = Mz^T z_k via fp8 DoubleRow matmuls; z_k's ones-row becomes a
    per-partition bias on the kz eviction (mzl).
  - Banded gram per 128-row block in fp8 DR + bf16 ones-row term, band mask
    via Pool memset(MASK) + DVE copy_predicated against a const predicate
    (no row-max subtraction needed: logits are bounded), exp w/ accum_out,
    bf16 PE transposes of the weights, fp8 banded weighting with a
    batch-gathered halo (vh_all, two partition-shift DMAs per rep).
  - The residual q/scale + bv rides the weighting PSUM via an
    identity-weights f32r matmul; the x2 evict multiplies by scale.
  - MLP: residual tiles stay SBUF-resident; LN2 stats batched upfront
    (one Sqrt act-table load), mm1/mm2 in fp8 DoubleRow, gelu evicts to
    fp8 pair tiles, mm2 software-pipelined one chunk behind mm1, b2 via an
    augmented ones-row matmul, residual add fused behind it.

All heavy matmuls are fp8e4 DoubleRow (0.5 PE cycles/row, 2 K-tiles per
instruction); LN rsqrt uses exp(-0.5 ln(v+eps)) so the K/Q phases share
one Act function table with the softmax exp (a table reload is 1.3us).
End-to-end relative error ~1.5e-2 (fp8 MLP dominates), verified on HW
against the fp32 reference. HW Gelu == erf gelu (verified).
"""

import os
from contextlib import ExitStack

import numpy as np

import concourse.bass as bass
import concourse.tile as tile
from concourse import bacc, mybir
from concourse.bass_utils import run_bass_kernel_spmd
from concourse.masks import make_identity
import ml_dtypes

f32 = mybir.dt.float32
f32r = mybir.dt.float32r
bf16 = mybir.dt.bfloat16
f8 = mybir.dt.float8e4
DR = mybir.MatmulPerfMode.DoubleRow
AF = mybir.ActivationFunctionType
ALU = mybir.AluOpType
AX = mybir.AxisListType

P = 128          # partitions
C = 512          # channels
H = 4 * C        # mlp hidden
EPS = 1e-5
NEG = -1e30
MZS = 128.0      # host scale on Mz (fp8 subnormal dodge); 1/MZS rides the exp
MASK = -1e4      # band-mask logit (pre-1/MZS); exp(-1e4/MZS) == 0

CT = C // P      # channel tiles (4)
HT = H // P      # mlp hidden tiles (16)

ts = bass.ts


def build_block_kernel(nc, N, KH, gelu_func=AF.Gelu, reps=1):
    """Emit the tile program. N = sequence length, KH = window size (odd)."""
    NT = N // P               # n tiles
    PADW = KH // 2            # 4
    W = P + KH - 1            # band tile width (136)
    NCH = N // 512            # 512-wide column chunks
    scale = C ** -0.5

    # ---- DRAM I/O ----
    dI = {}
    for nm, shp, dt in [
        ("qpe3", [P, N // P, C], bf16), ("kpe3", [P, N // P, C], bf16),
        ("qTb3", [P, N // P, C], f32r), ("keyC8", [P, CT, N], f8),
        ("Mz8", [P, CT, C + 512], f8), ("mzl", [P, CT + 1], f32),
        ("wv8", [P, CT, C], f8),
        ("W1p8", [P, CT, H], f8), ("w28", [P, HT, C], f8),
        ("c1t", [P, HT], f32), ("b2r", [1, C], f32r), ("onesr", [1, 512], f32r),
    ]:
        dI[nm] = nc.dram_tensor(nm, shp, dt, kind="ExternalInput").ap()
    outT = nc.dram_tensor("outT", [N, C], f32, kind="ExternalOutput").ap()

    with tile.TileContext(nc, pool_alloc_mode="queue") as tc, ExitStack() as ctx:
        # ---------- long-lived pools ----------
        psum = ctx.enter_context(tc.tile_pool(name="psum", bufs=6, space="PSUM"))
        _ctr = [0]

        def pt(shape, tag="ps", bufs=None, dt=f32):
            _ctr[0] += 1
            return psum.tile(shape, dt, tag=tag, name=f"pst{_ctr[0]}", bufs=bufs)

        const = ctx.enter_context(tc.tile_pool(name="const", bufs=1))
        stat_p = ctx.enter_context(tc.tile_pool(name="stat", bufs=8))
        work_p = ctx.enter_context(tc.tile_pool(name="work", bufs=6))

        ident = const.tile([P, P], f32)
        make_identity(nc, ident)
        ident_bf = const.tile([P, P], bf16)
        make_identity(nc, ident_bf)
        ident_r = const.tile([P, P], f32r)
        nc.vector.tensor_copy(ident_r, ident)
        ones_row = const.tile([1, 512], f32r)
        nc.sync.dma_start(ones_row, dI["onesr"])
        ones_bf = const.tile([1, 512], bf16)
        nc.vector.memset(ones_bf, 1.0)
        eps_col = const.tile([P, 1], f32)
        nc.vector.memset(eps_col, EPS)
        c1_sb = const.tile([P, HT], f32)
        nc.sync.dma_start(c1_sb, dI["c1t"])
        b2_sb = const.tile([1, C], f32r)
        nc.sync.dma_start(b2_sb, dI["b2r"])
        # band predicate: 1 where 0 <= f - p <= KH-1
        pred = const.tile([P, W], mybir.dt.int8)
        nc.vector.memset(pred, 1)
        nc.gpsimd.affine_select(out=pred, in_=pred, pattern=[[1, W]],
                                base=0, channel_multiplier=-1,
                                compare_op=ALU.is_ge, fill=0)
        nc.gpsimd.affine_select(out=pred, in_=pred, pattern=[[-1, W]],
                                base=KH - 1, channel_multiplier=1,
                                compare_op=ALU.is_ge, fill=0)

        # ---------- helpers ----------
        def emit_once():
            def load_quad(src, q4, tag, dt_=f32):
                """One DMA pulling 4 n-tiles of a [P, NT, C] DRAM tensor."""
                t = ldq_pool.tile([P, 4, C], dt_, tag=tag, bufs=2)
                nc.sync.dma_start(t, src[:, ts(q4, 4), :])
                return t

            def ln_quad_stats(srcs, mode="lnexp"):
                """Batched LN stats over 4 tiles -> (r4, nmr4).

                mode="lnexp" computes rsqrt as exp(-0.5*ln(v+eps)) so the Act
                table stays on the ln/exp set shared with the softmax exp
                (a table reload costs 1.3us); mode="sqrt" uses Sqrt+recip.
                """
                mv4 = stat_p.tile([P, 4, 2], f32, tag="mv4")
                for d, s in enumerate(srcs):
                    st6 = stat_p.tile([P, 6], f32, tag="st6")
                    nc.vector.bn_stats(st6, s)
                    nc.vector.bn_aggr(mv4[:, d, :], st6)
                r4 = stat_p.tile([P, 4], f32, tag="r4")
                if mode == "lnexp":
                    lv4 = stat_p.tile([P, 4], f32, tag="lv4")
                    nc.scalar.activation(lv4, mv4[:, :, 1], AF.Ln,
                                         bias=eps_col, scale=1.0)
                    nc.scalar.activation(r4, lv4, AF.Exp, bias=0.0, scale=-0.5)
                else:
                    sd4 = stat_p.tile([P, 4], f32, tag="sd4")
                    nc.scalar.activation(sd4, mv4[:, :, 1], AF.Sqrt,
                                         bias=eps_col, scale=1.0)
                    nc.vector.reciprocal(r4, sd4)
                nmr4 = stat_p.tile([P, 4], f32, tag="nmr4")
                nc.vector.scalar_tensor_tensor(out=nmr4, in0=mv4[:, :, 0],
                                               scalar=-1.0, in1=r4,
                                               op0=ALU.mult, op1=ALU.mult)
                return r4, nmr4

            def ln_quad_norm(srcs, r4, nmr4, odt):
                outs = []
                for d, s in enumerate(srcs):
                    xh = work_p.tile([P, C], odt, tag="xh")
                    # normalize on the (otherwise idle) Pool engine
                    nc.gpsimd.tensor_scalar(out=xh, in0=s,
                                            scalar1=r4[:, d:d + 1],
                                            scalar2=nmr4[:, d:d + 1],
                                            op0=ALU.mult, op1=ALU.add)
                    outs.append(xh)
                return outs

            def ln_quad(srcs, odt, mode="lnexp"):
                r4, nmr4 = ln_quad_stats(srcs, mode)
                return ln_quad_norm(srcs, r4, nmr4, odt)

            def ln_xhat_quad(src_a, q4):
                a = load_quad(src_a, q4, "ld_a", dt_=bf16)
                return ln_quad([a[:, d, :] for d in range(4)], odt=bf16)

            def transpose_quad(xh4, ct, dst, dst_cols, evict="vector"):
                """Transpose the ct-th c-block of 4 T-tiles into dst[:, dst_cols]
                (or into the AP `dst` itself when dst_cols is None)."""
                dt_ = xh4[0].dtype
                idn = ident_bf if dt_ == bf16 else ident
                ps_t = pt([P, 512], dt=dt_)
                for d, xh in enumerate(xh4):
                    nc.tensor.transpose(ps_t[:, ts(d, P)], xh[:, ts(ct, P)], idn)
                tgt = dst if dst_cols is None else dst[:, dst_cols]
                if evict == "vector":
                    nc.vector.tensor_copy(tgt, ps_t)
                elif evict == "scalar":
                    nc.scalar.copy(tgt, ps_t)
                elif evict == "pool":
                    nc.gpsimd.tensor_copy(tgt, ps_t)
                else:  # alternate
                    if ct % 2 == 0:
                        nc.vector.tensor_copy(tgt, ps_t)
                    else:
                        nc.scalar.copy(tgt, ps_t)

            # LIFO pool discipline: longer-lived pools allocated first.
            kz_pool = tc.alloc_tile_pool(name="kzp", bufs=1)
            aqr_pool = tc.alloc_tile_pool(name="aqrp", bufs=1)

            skip_kq = bool(os.environ.get("KSKIP_KQ"))
            skip_kz = bool(os.environ.get("KSKIP_KZ"))
            kz8 = [kz_pool.tile([P, 2, N], f8, name=f"kz{pr}")
                   for pr in range(2)]
            kz_last = kz_pool.tile([1, N], bf16)

            # --- MLP weight prefetch (DMAs interleaved into the Q phase)
            mlpw = tc.alloc_tile_pool(name="mlpw", bufs=1)
            W1p_sb = mlpw.tile([P, CT, H], f8, name="W1p8")
            w2_sb = mlpw.tile([P, HT, C], f8, name="w28")
            weight_dmas = []
            for kb in range(CT):
                weight_dmas.append((W1p_sb[:, kb, :], dI["W1p8"][:, kb, :]))
            for g4 in range(HT // 4):
                weight_dmas.append((w2_sb[:, ts(g4, 4), :],
                                    dI["w28"][:, ts(g4, 4), :]))

            # residual x2 tiles stay SBUF-resident through the MLP phase
            x2sb_pool = tc.alloc_tile_pool(name="x2sbp", bufs=1)
            x2sb = [x2sb_pool.tile([P, C], f32, name=f"x2sb{_nb}")
                    for _nb in range(NT)]

            wv_pool = tc.alloc_tile_pool(name="wvp", bufs=1)
            key_pool = tc.alloc_tile_pool(name="keyp", bufs=3)
            vt_pool = tc.alloc_tile_pool(name="vtp", bufs=1)
            x2_pool = tc.alloc_tile_pool(name="x2p", bufs=4)
            attn_p = tc.alloc_tile_pool(name="attnp", bufs=3)
            halo_p = tc.alloc_tile_pool(name="halop", bufs=1)
            wv8_sb = wv_pool.tile([P, CT, C], f8, name="wv8")
            nc.sync.dma_start(wv8_sb, dI["wv8"])
            # all 16 vT tiles live in one SBUF tensor; interior halo rows are
            # batch-gathered into vh_all with two partition-shift DMAs
            vt_all = vt_pool.tile([P, NT, C], f8, name="vt_all")
            vh_all = halo_p.tile([PADW, 2, NT - 1, C], f8, name="vh_all")

            # Mz is host-scaled by S (fp8-subnormal dodge); 1/S rides the exp.
            # ldq/mz/akr allocated last so their early release stays LIFO-valid.
            ldq_pool = tc.alloc_tile_pool(name="ldqp", bufs=2)
            mz_pool = tc.alloc_tile_pool(name="mzp", bufs=1)
            Mz8_sb = mz_pool.tile([P, CT, C + 512], f8, name="Mz8")
            nc.sync.dma_start(Mz8_sb, dI["Mz8"])
            mzl_sb = mz_pool.tile([P, CT + 1], f32, name="mzl")
            nc.sync.dma_start(mzl_sb, dI["mzl"])
            akr_pool = tc.alloc_tile_pool(name="akrp", bufs=1)
            akr8 = [akr_pool.tile([P, 2, N], f8, name=f"akr{pr}")
                    for pr in range(2)]

            def emit_v_chunk(ch):
                kc8 = key_pool.tile([P, CT, 512], f8, tag="keyc")
                nc.sync.dma_start(kc8, dI["keyC8"][:, :, ts(ch, 512)])
                for sub in range(4):
                    ps_v = pt([P, C])
                    for kp in range(2):
                        nc.tensor.matmul(ps_v,
                                         kc8[:, 2 * kp:2 * kp + 2, ts(sub, P)],
                                         wv8_sb[:, 2 * kp:2 * kp + 2, :],
                                         start=(kp == 0), stop=(kp == 1),
                                         perf_mode=DR)
                    if sub % 2 == 0:
                        nc.scalar.copy(vt_all[:, ch * 4 + sub, :], ps_v)
                    else:
                        nc.vector.tensor_copy(vt_all[:, ch * 4 + sub, :], ps_v)

            def emit_kz_chunk(ch):
                for m in range(CT + 1):
                    small = m == CT
                    m_sl = slice(C, C + 4) if small else ts(m, P)
                    ps_k = pt([4 if small else P, 512])
                    for kp in range(2):
                        nc.tensor.matmul(ps_k, Mz8_sb[:, 2 * kp:2 * kp + 2, m_sl],
                                         akr8[kp][:, :, ts(ch, 512)],
                                         start=(kp == 0), stop=(kp == 1),
                                         perf_mode=DR)
                    if small:
                        nc.scalar.activation(kz_last[:, ts(ch, 512)],
                                             ps_k[0:1, :],
                                             AF.Identity,
                                             bias=mzl_sb[0:1, CT:CT + 1],
                                             scale=1.0)
                    elif m % 2 == 0:
                        nc.scalar.activation(kz8[m // 2][:, m % 2, ts(ch, 512)],
                                             ps_k, AF.Identity,
                                             bias=mzl_sb[:, m:m + 1], scale=1.0)
                    else:
                        nc.vector.tensor_scalar_add(
                            kz8[m // 2][:, m % 2, ts(ch, 512)], ps_k,
                            mzl_sb[:, m:m + 1])

            # ===== merged K phase: per chunk: k-quad LN+transpose, vconv, kz ====
            for q4 in range(NT // 4):
                if not skip_kq:
                    quad = ln_xhat_quad(dI["kpe3"], q4)
                    for ct in range(CT):
                        transpose_quad(quad, ct,
                                       akr8[ct // 2][:, ct % 2, ts(q4, 512)],
                                       None, evict="alt")
                emit_v_chunk(q4)
                if not skip_kz:
                    emit_kz_chunk(q4)
            akr_pool.release()
            mz_pool.release()

            # batch halo gather, indexed so slot i serves block nb = i+1:
            # A side = rows 124..127 of tile i (= nb-1); C side = rows 0..3 of
            # tile i+2 (= nb+1). Both partition-shifted to rows 0..3.
            nc.sync.dma_start(vh_all[:, 0, :, :],
                              vt_all[P - PADW:P, 0:NT - 1, :])
            nc.sync.dma_start(vh_all[:, 1, 0:NT - 2, :],
                              vt_all[0:PADW, 2:NT, :])

            # ===== merged Q phase: per group: q-quad, then attention group =====
            aqr8 = [aqr_pool.tile([P, 2, N], f8, name=f"aqr{pr}")
                    for pr in range(2)]

            for g in range(NT // 4):
                if not skip_kq:
                    quad = ln_xhat_quad(dI["qpe3"], g)
                    for ct in range(CT):
                        transpose_quad(quad, ct,
                                       aqr8[ct // 2][:, ct % 2, ts(g, 512)],
                                       None, evict="alt")
                qtbq = x2_pool.tile([P, 4, C], f32r, tag="qtb",
                                    name=f"qtbq{g}", bufs=2)
                nc.sync.dma_start(qtbq, dI["qTb3"][:, ts(g, 4), :])
                # ---- pass 1: banded gram (fp8 DR) + band softmax
                wns = []
                for b in range(4):
                    nb = g * 4 + b
                    fl = PADW if nb == 0 else 0
                    fh = W - PADW if nb == NT - 1 else W
                    wvd = fh - fl
                    plo = nb * P - PADW + fl
                    ps_g = pt([P, W])
                    for kp in range(2):
                        nc.tensor.matmul(ps_g[:, fl:fh],
                                         aqr8[kp][:, :, ts(nb, P)],
                                         kz8[kp][:, :, plo:plo + wvd],
                                         start=(kp == 0), stop=False,
                                         perf_mode=DR)
                    nc.tensor.matmul(ps_g[:, fl:fh], ones_bf[:, 0:P],
                                     kz_last[:, plo:plo + wvd],
                                     start=False, stop=True)
                    gs = attn_p.tile([P, W], f32, tag="gs", bufs=5)
                    nc.gpsimd.memset(gs, MASK)
                    nc.vector.copy_predicated(gs[:, fl:fh], pred[:, fl:fh],
                                              ps_g[:, fl:fh])
                    ge = attn_p.tile([P, W], bf16, tag="ge", bufs=5)
                    esum = stat_p.tile([P, 1], f32, tag="esum")
                    nc.scalar.activation(ge, gs, AF.Exp, bias=0.0,
                                         scale=1.0 / MZS, accum_out=esum)
                    rsc = stat_p.tile([P, 1], f32, tag="rsc")
                    nc.vector.reciprocal(rsc, esum)
                    wn = attn_p.tile([P, W], bf16, tag="wn", bufs=5)
                    nc.gpsimd.tensor_scalar(out=wn, in0=ge, scalar1=rsc,
                                            scalar2=None, op0=ALU.mult)
                    wns.append(wn)
                # ---- pass 2: PE transposes (bf16 -> fp8), pair-batched evicts
                wbB4 = attn_p.tile([P, 4, P], f8, tag="wbB4", bufs=2,
                                   name=f"wbB4_{g}")
                wbac4 = attn_p.tile([PADW, 4, 2, P], f8, tag="wbac4", bufs=2,
                                    name=f"wbac4_{g}")
                psw2 = None
                for b in range(4):
                    nb = g * 4 + b
                    wn = wns[b]
                    if b % 2 == 0:
                        psw2 = pt([P, 2, 3 * P], tag="ps_w", bufs=2, dt=bf16)
                    ps_w = psw2[:, b % 2, :]
                    if nb > 0:
                        nc.tensor.transpose(ps_w[0:PADW, 0:P], wn[:, 0:PADW],
                                            ident_bf)
                    nc.tensor.transpose(ps_w[:, P:2 * P], wn[:, PADW:PADW + P],
                                        ident_bf)
                    if nb < NT - 1:
                        nc.tensor.transpose(ps_w[0:PADW, 2 * P:3 * P],
                                            wn[:, PADW + P:W], ident_bf)
                    if b % 2 == 1:
                        b0 = b - 1
                        nc.vector.tensor_copy(wbB4[:, b0:b + 1, :],
                                              psw2[:, :, P:2 * P])
                        if nb - 1 == 0:
                            # first block has no A piece
                            nc.scalar.copy(wbac4[:, b0, 1, :],
                                           psw2[0:PADW, 0, 2 * P:3 * P])
                            nc.scalar.copy(wbac4[:, b, :, :],
                                           psw2[0:PADW, 1, :].rearrange(
                                               "p (k c) -> p k c", k=3)[
                                                   :, 0:3:2, :])
                        elif nb == NT - 1:
                            # last block has no C piece
                            nc.scalar.copy(wbac4[:, b0, :, :],
                                           psw2[0:PADW, 0, :].rearrange(
                                               "p (k c) -> p k c", k=3)[
                                                   :, 0:3:2, :])
                            nc.scalar.copy(wbac4[:, b, 0, :],
                                           psw2[0:PADW, 1, 0:P])
                        else:
                            nc.scalar.copy(
                                wbac4[:, b0:b + 1, :, :],
                                psw2[0:PADW, :, :].rearrange(
                                    "p j (k c) -> p j k c", k=3)[:, :, 0:3:2, :])
                # ---- pass 3: banded weighting (fp8) + scaled residual
                for b in range(4):
                    nb = g * 4 + b
                    ps_a = pt([P, C])
                    nc.tensor.matmul(ps_a, wbB4[:, b, :], vt_all[:, nb, :],
                                     start=True, stop=False)
                    if nb == 0:
                        nc.tensor.matmul(ps_a, wbac4[:, b, 1, :],
                                         vt_all[0:PADW, 1, :],
                                         start=False, stop=False)
                    elif nb == NT - 1:
                        nc.tensor.matmul(ps_a, wbac4[:, b, 0, :],
                                         vh_all[:, 0, nb - 1, :],
                                         start=False, stop=False)
                    else:
                        nc.tensor.matmul(ps_a, wbac4[:, b, :, :],
                                         vh_all[:, :, nb - 1, :],
                                         start=False, stop=False,
                                         perf_mode=DR)
                    # residual rides PSUM: identity-weights matmul adds
                    # qTb/scale; the evict then just multiplies by scale.
                    nc.tensor.matmul(ps_a, ident_r, qtbq[:, b, :],
                                     start=False, stop=True)
                    if b % 2 == 0:
                        nc.scalar.mul(x2sb[nb], ps_a, scale)
                    else:
                        nc.vector.tensor_scalar_mul(x2sb[nb], ps_a, scale)
                # interleave a slice of the MLP weight prefetch
                nper = (len(weight_dmas) + NT // 4 - 1) // (NT // 4)
                for dst, src in weight_dmas[g * nper:(g + 1) * nper]:
                    nc.sync.dma_start(dst, src)

            ldq_pool.release()
            halo_p.release()
            attn_p.release()
            x2_pool.release()
            vt_pool.release()
            key_pool.release()
            wv_pool.release()

            if os.environ.get("KSKIP_MLP"):
                x2sb_pool.release()
                mlpw.release()
                aqr_pool.release()
                kz_pool.release()
                return
            # ================= MLP phase =================
            xh2c_pool = tc.alloc_tile_pool(name="xh2cp", bufs=8)
            hg_pool = tc.alloc_tile_pool(name="hgp", bufs=24)
            fin_pool = tc.alloc_tile_pool(name="finp", bufs=4)

            def emit_mm2(hgp, x2c, ch):
                # mm2 (fp8 DoubleRow, T-layout out) + b2 aug + residual
                for sub in range(4):
                    nb = ch * 4 + sub
                    ps_o = pt([P, C])
                    for k2 in range(HT // 2):
                        nc.tensor.matmul(ps_o, hgp[k2][:, :, ts(sub, P)],
                                         w2_sb[:, 2 * k2:2 * k2 + 2, :],
                                         start=(k2 == 0), stop=False,
                                         perf_mode=DR)
                    nc.tensor.matmul(ps_o, ones_row[:, 0:P], b2_sb,
                                     start=False, stop=True)
                    fin = fin_pool.tile([P, C], f32, tag="fin")
                    nc.vector.tensor_add(fin, ps_o, x2c[sub])
                    nc.sync.dma_start(outT[ts(nb, P), :], fin)

            # all LN2 stats upfront: one Sqrt table load, then gelu-only
            ln2_stats = [ln_quad_stats(x2sb[c4 * 4:c4 * 4 + 4], mode="sqrt")
                         for c4 in range(NCH)]
            pend = None  # software pipeline: mm2 of ch runs during ch+1
            for ch in range(NCH):
                x2c = x2sb[ch * 4:ch * 4 + 4]
                xh2 = ln_quad_norm(x2c, *ln2_stats[ch], odt=bf16)
                xh2c = xh2c_pool.tile([P, CT, 512], f8, tag="xh2c")
                for ct in range(CT):
                    transpose_quad(xh2, ct, xh2c[:, ct, :], None, evict="alt")
                # mm1 (fp8 DoubleRow) + gelu -> fp8 pair tiles
                hgp = [hg_pool.tile([P, 2, 512], f8, tag="hg", name=f"hgp{_m}")
                       for _m in range(HT // 2)]
                for m in range(HT):
                    ps_h = pt([P, 512])
                    for kp in range(CT // 2):
                        nc.tensor.matmul(
                            ps_h, W1p_sb[:, 2 * kp:2 * kp + 2, ts(m, P)],
                            xh2c[:, 2 * kp:2 * kp + 2, :],
                            start=(kp == 0), stop=(kp == CT // 2 - 1),
                            perf_mode=DR)
                    nc.scalar.activation(hgp[m // 2][:, m % 2, :], ps_h,
                                         gelu_func, bias=c1_sb[:, m:m + 1],
                                         scale=1.0)
                if pend is not None:
                    emit_mm2(*pend)
                pend = (hgp, x2c, ch)
            emit_mm2(*pend)

            fin_pool.release()
            hg_pool.release()
            xh2c_pool.release()
            x2sb_pool.release()
            mlpw.release()
            aqr_pool.release()
            kz_pool.release()
            del x2sb

        # Timing-only (reps>1): replay rep-1's exact SBUF ring layout for
        # every later rep (same pool names -> same addresses); the
        # released_zones overlap-deps then serialize rep k+1 on rep k.
        # No effect on the graded reps=1 build.
        if reps > 1:
            _memo = {}
            _orig_qa = tc._queue_alloc

            def _qa(pool):
                if pool.name in _memo:
                    base, end = _memo[pool.name]
                    pool._ring_addr = (base, end)
                    rb, re_, _h = tc._queue_ring
                    tc._queue_ring = (rb, re_, end)
                    return base, end
                r = _orig_qa(pool)
                _memo[pool.name] = r
                return r

            tc._queue_alloc = _qa

        for _rep in range(reps):
            emit_once()

    return dI, outT


_CACHE = {}


def _pin_act_tables(arch):
    """Make Ln/Exp first-match into the set that holds BOTH (set 6,
    natural_log_exp_and_others), so LN-rsqrt (exp(-0.5 ln v)) and the
    softmax exp share one table: a table reload costs 1.3us on Act."""
    from concourse.hw_specs import get_activation_tables
    tabs = get_activation_tables(arch)
    for name, funcs in tabs.items():
        if name == "natural_log_exp_and_others":
            break
        funcs.discard(AF.Exp)
        funcs.discard(AF.Ln)


def _get_compiled(N, KH, gelu_func=AF.Gelu, reps=1):
    key = (N, KH, str(gelu_func), reps)
    if key not in _CACHE:
        nc = bacc.Bacc("TRN2", target_bir_lowering=False, debug=False,
                       enable_asserts=False)
        _pin_act_tables(nc.m.arch)
        build_block_kernel(nc, N, KH, gelu_func, reps=reps)
        nc.compile()
        _CACHE[key] = nc
    return _CACHE[key]


def host_prep(inputs, N, KH):
    """Fold weights and build the per-core input maps."""
    q = np.asarray(inputs["query"], np.float32)
    k = np.asarray(inputs["key"], np.float32)
    qe = np.asarray(inputs["query_embed"], np.float32)
    ke = np.asarray(inputs["key_embed"], np.float32)
    wq = np.asarray(inputs["wq"], np.float32)
    bq = np.asarray(inputs["bq"], np.float32)
    wk = np.asarray(inputs["wk"], np.float32)
    bk = np.asarray(inputs["bk"], np.float32)
    wv = np.asarray(inputs["wv"], np.float32)
    bv = np.asarray(inputs["bv"], np.float32)
    g = np.asarray(inputs["g_norm"], np.float32)
    b = np.asarray(inputs["b_norm"], np.float32)
    g2 = np.asarray(inputs["g_norm2"], np.float32)
    b2n = np.asarray(inputs["b_norm2"], np.float32)
    w1 = np.asarray(inputs["w1"], np.float32)
    b1 = np.asarray(inputs["b1"], np.float32)
    w2 = np.asarray(inputs["w2"], np.float32)
    b2 = np.asarray(inputs["b2"], np.float32)

    Bsz = q.shape[0]
    scale = C ** -0.5

    Aq = np.concatenate([wq * g[None, :], (wq @ b + bq)[:, None]], axis=1)
    Ak = np.concatenate([wk * g[None, :], (wk @ b + bk)[:, None]], axis=1)
    Mz = (Ak.T @ Aq) * MZS

    W1p = w1 * g2[:, None]
    c1 = b2n @ w1 + b1
    c1t = np.ascontiguousarray(c1.reshape(HT, P).T)
    shared = {
        "Mz8": np.ascontiguousarray(np.pad(
            Mz[0:C], ((0, 0), (0, 511))).reshape(CT, P, C + 512).transpose(
                1, 0, 2)).astype(ml_dtypes.float8_e4m3),
        "mzl": np.ascontiguousarray(np.concatenate(
            [Mz[C, 0:C].reshape(CT, P).T,
             np.full((P, 1), Mz[C, C], np.float32)], axis=1)),
        "wv8": np.ascontiguousarray(
            wv.T.reshape(CT, P, C).transpose(1, 0, 2)).astype(
                ml_dtypes.float8_e4m3),
        "W1p8": np.ascontiguousarray(
            W1p.reshape(CT, P, H).transpose(1, 0, 2)).astype(
                ml_dtypes.float8_e4m3),
        "w28": np.ascontiguousarray(
            w2.reshape(HT, P, C).transpose(1, 0, 2)).astype(
                ml_dtypes.float8_e4m3),
        "c1t": c1t,
        "b2r": np.ascontiguousarray(b2[None, :]),
        "onesr": np.ones((1, 512), np.float32),
    }
    N_ = q.shape[2]
    NT_ = N_ // P

    def t3(x):  # [C, N] -> [P, NT, C] partition-major n-tiles
        return np.ascontiguousarray(
            x.T.reshape(NT_, P, C).transpose(1, 0, 2))

    in_maps = []
    for i in range(Bsz):
        m = dict(shared)
        m["qpe3"] = t3(q[i] + qe[i]).astype(ml_dtypes.bfloat16)
        m["kpe3"] = t3(k[i] + ke[i]).astype(ml_dtypes.bfloat16)
        m["qTb3"] = t3(q[i] / scale + bv[:, None])
        m["keyC8"] = np.ascontiguousarray(
            k[i].reshape(CT, P, -1).transpose(1, 0, 2)).astype(
                ml_dtypes.float8_e4m3)
        in_maps.append(m)
    return in_maps


def kernel(**inputs):
    q = np.asarray(inputs["query"])
    Bsz, Cin, N = q.shape
    assert Cin == C, f"built for C={C}"
    KH = int(inputs["kH"])
    nc = _get_compiled(N, KH)
    in_maps = host_prep(inputs, N, KH)
    core_ids = list(range(len(in_maps)))
    res = run_bass_kernel_spmd(nc, in_maps, core_ids)
    out = np.stack([np.ascontiguousarray(r["outT"].T) for r in res.results], axis=0)
    return out.astype(np.float32)


if __name__ == "__main__":
    _get_compiled(2048, 9)
    print("built + compiled OK")

